# revision 40
# baseline (speedup 1.0000x reference)
"""Trainium2 kernel for nn_EulerBias: exact Riemann-solver bias field.

Structure:
  * Host (numpy, float32): the K-interface Newton solve (tiny: B x 63) ->
    wave speeds, then per-batch coefficient matrices for the device stage.
  * Device (8 NeuronCores, batch-parallel, 2 batches/core): for every query
    point q the bias over the 64 segment columns is

        out[q,k] = min(T1[q,k],0) + min(T2[q,k],0)

    where T1/T2 are affine in (u,it,1) = (x*it, 1/(t+eps), 1) with per-k
    coefficients -> one small-contraction (Kc=12) float32r matmul on
    TensorE produces T1||T2 for 512 queries per instruction (f32r streams
    at 1 cycle/row for N>=256 vs 4 for plain fp32 — tf32-like multiply
    precision, rel err ~2.5e-4 vs the 2e-2 gate); ScalarE computes
    relu(-T2); VectorE fuses min(T1,0) - relu(-T2) in one op; SP issues one
    contiguous 1MB store per supertile-pair while the qd loads ride SWDGE
    on the otherwise-idle Pool engine so a store's semaphore wait cannot
    head-of-line block them.

The kernel is store-bound: 16.78MB of f32 output per core drains at
~336 GB/s effective, within ~5% of the ~358 GB/s per-core HBM roofline
(measured ~51-53us/exec vs the 196.7us session baseline).

Masked columns (pieces_mask == 0) are encoded in the coefficients
(T1 = -1e9, T2 = +1e30) so no separate mask pass is needed. Assumes
pieces_mask >= 0 (it is a 0/1 mask; the harness fills ones).
"""

import numpy as np

GAMMA = np.float32(1.4)
EPS = np.float32(1e-6)
N_NEWTON = 20
B, K, NT, NX = 16, 64, 128, 256
NQ = NT * NX            # 32768 queries per batch
N_CORES = 8
B_PER_CORE = B // N_CORES
# device tiling
CHUNK = 128             # queries per output-partition group
GROUPS = 4              # chunks per matmul (stationary rows = 3*GROUPS = 12)
SUPER = 4               # matmuls per supertile (PSUM banks)
Q_SUPER = CHUNK * GROUPS * SUPER          # 2048 queries per supertile
ST_PER_BATCH = NQ // Q_SUPER              # 16
BIG = np.float32(1e30)
NEGBIG = np.float32(-1e9)
SP_PER_STORE = 1        # supertile-pairs (1MB output) per store DMA
DUAL_RING = False      # measured: SWDGE-odd stores are ~4us slower on HW; keep all on SP
OT_BUFS = None          # None -> per-granularity default

_COMPILED = {}


def _f32(x):
    return np.asarray(x, dtype=np.float32)


def _host_wave_speeds(xs, ks, ks_v, ks_p):
    """Mirror of reference.py's f32 Newton solve, in numpy float32."""
    gm1 = np.float32(GAMMA - 1.0)
    gp1 = np.float32(GAMMA + 1.0)
    exp_rare = np.float32(gm1 / (2.0 * GAMMA))

    def clip_lo(v, lo=EPS):
        return np.maximum(v, lo)

    rho_L, rho_R = ks[:, :-1], ks[:, 1:]
    u_L, u_R = ks_v[:, :-1], ks_v[:, 1:]
    p_L, p_R = ks_p[:, :-1], ks_p[:, 1:]

    def sound(rho, p):
        return np.sqrt(clip_lo(GAMMA * p / clip_lo(rho)))

    c_L, c_R = sound(rho_L, p_L), sound(rho_R, p_R)
    A_L = np.float32(2.0) / (gp1 * clip_lo(rho_L))
    A_R = np.float32(2.0) / (gp1 * clip_lo(rho_R))
    B_L = gm1 / gp1 * p_L
    B_R = gm1 / gp1 * p_R

    def wave_f_df(p, p_K, A_K, B_K, c_K):
        denom = clip_lo(p + B_K)
        sqrt_AoD = np.sqrt(clip_lo(A_K / denom))
        f_shock = (p - p_K) * sqrt_AoD
        df_shock = sqrt_AoD * (np.float32(1.0) - (p - p_K) / (np.float32(2.0) * denom))
        p_ratio = clip_lo(p / clip_lo(p_K))
        f_rare = np.float32(2.0) * c_K / gm1 * (p_ratio ** exp_rare - np.float32(1.0))
        df_rare = c_K / (GAMMA * clip_lo(p_K)) * p_ratio ** np.float32(-gp1 / (2.0 * GAMMA))
        is_shock = p > p_K
        return np.where(is_shock, f_shock, f_rare), np.where(is_shock, df_shock, df_rare)

    p0 = clip_lo(((c_L + c_R - gm1 / np.float32(2.0) * (u_R - u_L))
                  / (c_L / clip_lo(p_L) ** exp_rare + c_R / clip_lo(p_R) ** exp_rare))
                 ** np.float32(1.0 / exp_rare))
    p_star = p0
    for _ in range(N_NEWTON):
        f_L, df_L = wave_f_df(p_star, p_L, A_L, B_L, c_L)
        f_R, df_R = wave_f_df(p_star, p_R, A_R, B_R, c_R)
        residual = f_L + f_R + (u_R - u_L)
        jacobian = clip_lo(df_L + df_R)
        p_star = clip_lo(p_star - residual / jacobian)

    gp1_o_2g = np.float32(gp1 / (2.0 * GAMMA))
    sigma_1 = u_L - c_L * np.sqrt(clip_lo(np.float32(1.0) + gp1_o_2g * (p_star / clip_lo(p_L) - np.float32(1.0))))
    speed_left = np.where(p_star > p_L, sigma_1, u_L - c_L)
    sigma_3 = u_R + c_R * np.sqrt(clip_lo(np.float32(1.0) + gp1_o_2g * (p_star / clip_lo(p_R) - np.float32(1.0))))
    speed_right = np.where(p_star > p_R, sigma_3, u_R + c_R)
    return speed_left.astype(np.float32), speed_right.astype(np.float32)


def _host_coef(xs, mask, sl, sr):
    """Per-batch [12, 512] moving-operand coefficient matrices.

    psum col n = 64*j + k      (j = chunk-in-group) -> T1 = -m*relu-arg form
    psum col n = 256 + 64*j + k                     -> T2
    contraction rows 3j+(0,1,2) multiply (u, it, 1) of chunk j.
    """
    xd = xs[:, 1:K]                      # (B, 63)
    m = mask.astype(np.float32)          # (B, 64)
    act = m != 0

    # T1 = -m*u + m*xd*it + m*sr   (k < 63);  col 63 -> +BIG;  masked -> -1e9 const
    Wu1 = np.zeros((B, K), np.float32)
    Wi1 = np.zeros((B, K), np.float32)
    Wc1 = np.zeros((B, K), np.float32)
    Wu1[:, :63] = -m[:, :63]
    Wi1[:, :63] = m[:, :63] * xd
    Wc1[:, :63] = m[:, :63] * sr
    Wc1[:, 63] = BIG
    Wu1[~act] = 0.0
    Wi1[~act] = 0.0
    Wc1[~act] = NEGBIG

    # T2 = m*u - m*xd[k-1]*it - m*sl[k-1] (k >= 1); col 0 or masked -> +BIG
    # (so min(T2,0) = -m*relu(sl[k-1] - xi[k-1]))
    Wu2 = np.zeros((B, K), np.float32)
    Wi2 = np.zeros((B, K), np.float32)
    Wc2 = np.zeros((B, K), np.float32)
    Wu2[:, 1:] = m[:, 1:]
    Wi2[:, 1:] = -m[:, 1:] * xd
    Wc2[:, 1:] = -m[:, 1:] * sl
    Wc2[:, 0] = BIG
    Wu2[~act] = 0.0
    Wi2[~act] = 0.0
    Wc2[~act] = BIG

    coef = np.zeros((B, 3 * GROUPS, 512), np.float32)
    for j in range(GROUPS):
        c1 = slice(64 * j, 64 * j + 64)
        c2 = slice(256 + 64 * j, 256 + 64 * j + 64)
        coef[:, 3 * j + 0, c1] = Wu1
        coef[:, 3 * j + 1, c1] = Wi1
        coef[:, 3 * j + 2, c1] = Wc1
        coef[:, 3 * j + 0, c2] = Wu2
        coef[:, 3 * j + 1, c2] = Wi2
        coef[:, 3 * j + 2, c2] = Wc2
    return coef


def _host_qdata(t_coords, x_coords):
    """(B, 12, ST/2 * 1024) stationary operands: row 3j+(0,1,2) = (u, it, 1),
    column sp*1024 + 512*h + 128*g + m.

    Query assignment q(sp, h, g, j, m) = sp*4096 + h*2048 + m*16 + g*4 + j, so
    each supertile-pair's partition-major store walk (m, h, (g j), k) writes
    one monotonically contiguous 1MB HBM range. The whole batch's qd is one
    flat 384KB contiguous load."""
    it = np.float32(1.0) / (t_coords.reshape(B, NQ) + EPS)
    u = x_coords.reshape(B, NQ) * it

    def lay(v):
        # (b, sp, h, m, g, j) -> [b, sp, j, (h, g, m)]
        v = v.reshape(B, ST_PER_BATCH // 2, 2, CHUNK, SUPER, GROUPS)
        return np.transpose(v, (0, 1, 5, 2, 4, 3)).reshape(
            B, ST_PER_BATCH // 2, GROUPS, 2 * SUPER * CHUNK)

    qd = np.empty((B, ST_PER_BATCH // 2, 3 * GROUPS, 2 * SUPER * CHUNK), np.float32)
    qd[:, :, 0::3, :] = lay(u)
    qd[:, :, 1::3, :] = lay(it)
    qd[:, :, 2::3, :] = 1.0
    # (b, sp, row, col) -> (b, row, sp*1024 + col)
    return np.ascontiguousarray(qd.transpose(0, 2, 1, 3).reshape(
        B, 3 * GROUPS, (ST_PER_BATCH // 2) * 2 * SUPER * CHUNK))


def _build_nc(repeat=1, sp_per_store=SP_PER_STORE):
    """repeat>1 compiles a timing variant whose body re-runs the full
    load->compute->store pipeline `repeat` times (same I/O tensors), so the
    slope over `repeat` isolates steady-state per-execution device time
    from the axon per-dispatch RPC overhead. sp_per_store: supertile-pairs
    (1MB of output) per store DMA."""
    import concourse.bacc as bacc
    import concourse.mybir as mybir
    import concourse.tile as tile

    nc = bacc.Bacc(None, target_bir_lowering=False, debug=False)
    # float32r: same f32 bytes, but the PE streams it at 1 cycle/row for
    # moving dims >= 256 (vs 4 cycles/row for plain fp32's two half-speed
    # passes) — 4x matmul throughput at (reduced, tf32-like) multiply
    # precision, well inside the 2e-2 gate. (bf16 operands were measured
    # identical in speed — loads hide in DMA gaps — so f32r keeps the 78x
    # precision margin for free.)
    f32r = mybir.dt.float32r
    qd_d = nc.declare_dram_parameter(
        "qd", [B_PER_CORE, 3 * GROUPS,
               (ST_PER_BATCH // 2) * 2 * SUPER * CHUNK],
        f32r, isOutput=False)
    cf_d = nc.declare_dram_parameter(
        "cf", [B_PER_CORE, 3 * GROUPS, 512], f32r, isOutput=False)
    out_d = nc.declare_dram_parameter(
        "out", [B_PER_CORE, NQ, K], mybir.dt.float32, isOutput=True)

    f32 = mybir.dt.float32
    with tile.TileContext(nc) as tc:
        with (
            tc.tile_pool(name="cf", bufs=1) as cfp,
            tc.tile_pool(name="qd", bufs=6) as qdp,
            tc.tile_pool(name="ps", bufs=4, space="PSUM") as psp,
            tc.tile_pool(name="p2", bufs=6) as p2p,
            tc.tile_pool(name="ot",
                         bufs=(OT_BUFS or
                               {1: 8, 2: 6, 4: 3, 8: 2}[sp_per_store])) as otp,
        ):
            cft = []
            for b in range(B_PER_CORE):
                c = cfp.tile([3 * GROUPS, 512], f32r, tag=f"cf{b}")
                nc.gpsimd.dma_start(c[:], cf_d[b])
                cft.append(c)
            for rep in range(repeat):
              for b in range(B_PER_CORE):
                for spg in range(ST_PER_BATCH // 2 // sp_per_store):
                    ot = otp.tile(
                        [128, sp_per_store * 2 * SUPER, 256], f32)
                    for si in range(sp_per_store):
                        sp = spg * sp_per_store + si
                        # per-sp 48KB loads (4KB descriptors) ride SWDGE on
                        # the otherwise-idle Pool engine so a store's
                        # semaphore wait on SP can't head-of-line block them
                        # (transfers still share the one SDMA array)
                        o0 = sp * 2 * SUPER * CHUNK
                        qdt = qdp.tile([3 * GROUPS, 2 * SUPER * CHUNK], f32r)
                        nc.gpsimd.dma_start(
                            qdt[:], qd_d[b][:, o0:o0 + 2 * SUPER * CHUNK])
                        for h in range(2):
                            # two 2-bank PSUM tiles per half: deeper PE
                            # pipelining and finer ACT/STT quanta that
                            # overlap the stores.
                            for half in range(2):
                                ps = psp.tile([128, 2, 512], f32)
                                for g2 in range(2):
                                    g = half * 2 + g2
                                    nc.tensor.matmul(
                                        ps[:, g2, :],
                                        qdt[:, 512 * h + 128 * g:
                                            512 * h + 128 * (g + 1)],
                                        cft[b][:],
                                        start=True, stop=True,
                                    )
                                p2 = p2p.tile([128, 2, 256], f32)
                                nc.scalar.activation(
                                    p2[:], ps[:, :, 256:512],
                                    mybir.ActivationFunctionType.Relu,
                                    scale=-1.0)
                                c0 = 8 * si + 4 * h + 2 * half
                                nc.vector.scalar_tensor_tensor(
                                    out=ot[:, c0:c0 + 2],
                                    in0=ps[:, :, 0:256], scalar=0.0,
                                    in1=p2[:],
                                    op0=mybir.AluOpType.min,
                                    op1=mybir.AluOpType.subtract)
                    # one contiguous sp_per_store-MB store per group
                    q0 = spg * sp_per_store * 2 * Q_SUPER
                    sh = sp_per_store * 2
                    dst = out_d[b, q0:q0 + sh * Q_SUPER, :].rearrange(
                        "(s m c) k -> m s c k", s=sh, c=SUPER * GROUPS)
                    src = ot[:].rearrange(
                        "m (s g) (j k) -> m s (g j) k", s=sh, k=K)
                    eng = (nc.gpsimd if (DUAL_RING and spg % 2) else nc.sync)
                    eng.dma_start(dst, src)
    nc.compile()
    return nc


def _get_compiled(repeat=1):
    key = (repeat, SP_PER_STORE, DUAL_RING, OT_BUFS)
    if key not in _COMPILED:
        _COMPILED[key] = _build_nc(repeat, SP_PER_STORE)
    return _COMPILED[key]


def device_in_maps(inputs):
    xs = _f32(inputs["xs"])
    ks = _f32(inputs["ks"])
    ks_v = _f32(inputs["ks_v"])
    ks_p = _f32(inputs["ks_p"])
    mask = _f32(inputs["pieces_mask"])
    t_coords = _f32(inputs["t_coords"])
    x_coords = _f32(inputs["x_coords"])

    sl, sr = _host_wave_speeds(xs, ks, ks_v, ks_p)
    coef = _host_coef(xs, mask, sl, sr)
    qd = _host_qdata(t_coords, x_coords)
    return [
        {
            "qd": np.ascontiguousarray(qd[c * B_PER_CORE:(c + 1) * B_PER_CORE]),
            "cf": np.ascontiguousarray(coef[c * B_PER_CORE:(c + 1) * B_PER_CORE]),
        }
        for c in range(N_CORES)
    ]


def run(inputs, trace=False):
    from concourse.bass_utils import run_bass_kernel_spmd

    nc = _get_compiled()
    in_maps = device_in_maps(inputs)
    res = None
    for attempt in range(3):
        try:
            res = run_bass_kernel_spmd(
                nc, in_maps, core_ids=list(range(N_CORES)), trace=trace)
            break
        except Exception:
            if attempt == 2:
                raise
            import time as _time
            _time.sleep(2.0)
    out = np.empty((B, NT, NX, K), np.float32)
    for c in range(N_CORES):
        out[c * B_PER_CORE:(c + 1) * B_PER_CORE] = (
            res.results[c]["out"].reshape(B_PER_CORE, NT, NX, K))
    return out, res


def kernel(**inputs):
    out, _ = run(inputs, trace=False)
    return out



# revision 44
# speedup vs baseline: 1.0090x; 1.0090x over previous
"""Trainium2 kernel for nn_EulerBias: exact Riemann-solver bias field.

Structure:
  * Host (numpy, float32): the K-interface Newton solve (tiny: B x 63) ->
    wave speeds, then per-batch coefficient matrices for the device stage.
  * Device (8 NeuronCores, batch-parallel, 2 batches/core): for every query
    point q the bias over the 64 segment columns is

        out[q,k] = min(T1[q,k],0) + min(T2[q,k],0)

    where T1/T2 are affine in (u,it,1) = (x*it, 1/(t+eps), 1) with per-k
    coefficients -> one small-contraction (Kc=12) float32r matmul on
    TensorE produces T1||T2 for 512 queries per instruction (f32r streams
    at 1 cycle/row for N>=256 vs 4 for plain fp32 — tf32-like multiply
    precision, rel err ~2.5e-4 vs the 2e-2 gate); ScalarE computes
    relu(-T2); VectorE fuses min(T1,0) - relu(-T2) in one op; SP issues one
    contiguous 1MB store per supertile-pair while the qd loads ride SWDGE
    on the otherwise-idle Pool engine so a store's semaphore wait cannot
    head-of-line block them.

The kernel is store-bound: 16.78MB of f32 output per core drains at
~336 GB/s effective, within ~5% of the ~358 GB/s per-core HBM roofline
(measured ~51-53us/exec vs the 196.7us session baseline).

Masked columns (pieces_mask == 0) are encoded in the coefficients
(T1 = -1e9, T2 = +1e30) so no separate mask pass is needed. Assumes
pieces_mask >= 0 (it is a 0/1 mask; the harness fills ones).
"""

import numpy as np

GAMMA = np.float32(1.4)
EPS = np.float32(1e-6)
N_NEWTON = 20
B, K, NT, NX = 16, 64, 128, 256
NQ = NT * NX            # 32768 queries per batch
N_CORES = 8
B_PER_CORE = B // N_CORES
# device tiling
CHUNK = 128             # queries per output-partition group
GROUPS = 4              # chunks per matmul (stationary rows = 3*GROUPS = 12)
SUPER = 4               # matmuls per supertile (PSUM banks)
Q_SUPER = CHUNK * GROUPS * SUPER          # 2048 queries per supertile
ST_PER_BATCH = NQ // Q_SUPER              # 16
BIG = np.float32(1e30)
NEGBIG = np.float32(-1e9)
SP_PER_STORE = 1        # supertile-pairs (1MB output) per store DMA
DUAL_RING = False      # measured: SWDGE-odd stores are ~4us slower on HW; keep all on SP
OT_BUFS = None          # None -> per-granularity default
LOADS_ON_SP = False     # issue qd/cf loads on SP HWDGE instead of Pool SWDGE

_COMPILED = {}


def _f32(x):
    return np.asarray(x, dtype=np.float32)


def _host_wave_speeds(xs, ks, ks_v, ks_p):
    """Mirror of reference.py's f32 Newton solve, in numpy float32."""
    gm1 = np.float32(GAMMA - 1.0)
    gp1 = np.float32(GAMMA + 1.0)
    exp_rare = np.float32(gm1 / (2.0 * GAMMA))

    def clip_lo(v, lo=EPS):
        return np.maximum(v, lo)

    rho_L, rho_R = ks[:, :-1], ks[:, 1:]
    u_L, u_R = ks_v[:, :-1], ks_v[:, 1:]
    p_L, p_R = ks_p[:, :-1], ks_p[:, 1:]

    def sound(rho, p):
        return np.sqrt(clip_lo(GAMMA * p / clip_lo(rho)))

    c_L, c_R = sound(rho_L, p_L), sound(rho_R, p_R)
    A_L = np.float32(2.0) / (gp1 * clip_lo(rho_L))
    A_R = np.float32(2.0) / (gp1 * clip_lo(rho_R))
    B_L = gm1 / gp1 * p_L
    B_R = gm1 / gp1 * p_R

    def wave_f_df(p, p_K, A_K, B_K, c_K):
        denom = clip_lo(p + B_K)
        sqrt_AoD = np.sqrt(clip_lo(A_K / denom))
        f_shock = (p - p_K) * sqrt_AoD
        df_shock = sqrt_AoD * (np.float32(1.0) - (p - p_K) / (np.float32(2.0) * denom))
        p_ratio = clip_lo(p / clip_lo(p_K))
        f_rare = np.float32(2.0) * c_K / gm1 * (p_ratio ** exp_rare - np.float32(1.0))
        df_rare = c_K / (GAMMA * clip_lo(p_K)) * p_ratio ** np.float32(-gp1 / (2.0 * GAMMA))
        is_shock = p > p_K
        return np.where(is_shock, f_shock, f_rare), np.where(is_shock, df_shock, df_rare)

    p0 = clip_lo(((c_L + c_R - gm1 / np.float32(2.0) * (u_R - u_L))
                  / (c_L / clip_lo(p_L) ** exp_rare + c_R / clip_lo(p_R) ** exp_rare))
                 ** np.float32(1.0 / exp_rare))
    p_star = p0
    for _ in range(N_NEWTON):
        f_L, df_L = wave_f_df(p_star, p_L, A_L, B_L, c_L)
        f_R, df_R = wave_f_df(p_star, p_R, A_R, B_R, c_R)
        residual = f_L + f_R + (u_R - u_L)
        jacobian = clip_lo(df_L + df_R)
        p_star = clip_lo(p_star - residual / jacobian)

    gp1_o_2g = np.float32(gp1 / (2.0 * GAMMA))
    sigma_1 = u_L - c_L * np.sqrt(clip_lo(np.float32(1.0) + gp1_o_2g * (p_star / clip_lo(p_L) - np.float32(1.0))))
    speed_left = np.where(p_star > p_L, sigma_1, u_L - c_L)
    sigma_3 = u_R + c_R * np.sqrt(clip_lo(np.float32(1.0) + gp1_o_2g * (p_star / clip_lo(p_R) - np.float32(1.0))))
    speed_right = np.where(p_star > p_R, sigma_3, u_R + c_R)
    return speed_left.astype(np.float32), speed_right.astype(np.float32)


def _host_coef(xs, mask, sl, sr):
    """Per-batch [12, 512] moving-operand coefficient matrices.

    psum col n = 64*j + k      (j = chunk-in-group) -> T1 = -m*relu-arg form
    psum col n = 256 + 64*j + k                     -> T2
    contraction rows 3j+(0,1,2) multiply (u, it, 1) of chunk j.
    """
    xd = xs[:, 1:K]                      # (B, 63)
    m = mask.astype(np.float32)          # (B, 64)
    act = m != 0

    # T1 = -m*u + m*xd*it + m*sr   (k < 63);  col 63 -> +BIG;  masked -> -1e9 const
    Wu1 = np.zeros((B, K), np.float32)
    Wi1 = np.zeros((B, K), np.float32)
    Wc1 = np.zeros((B, K), np.float32)
    Wu1[:, :63] = -m[:, :63]
    Wi1[:, :63] = m[:, :63] * xd
    Wc1[:, :63] = m[:, :63] * sr
    Wc1[:, 63] = BIG
    Wu1[~act] = 0.0
    Wi1[~act] = 0.0
    Wc1[~act] = NEGBIG

    # T2 = m*u - m*xd[k-1]*it - m*sl[k-1] (k >= 1); col 0 or masked -> +BIG
    # (so min(T2,0) = -m*relu(sl[k-1] - xi[k-1]))
    Wu2 = np.zeros((B, K), np.float32)
    Wi2 = np.zeros((B, K), np.float32)
    Wc2 = np.zeros((B, K), np.float32)
    Wu2[:, 1:] = m[:, 1:]
    Wi2[:, 1:] = -m[:, 1:] * xd
    Wc2[:, 1:] = -m[:, 1:] * sl
    Wc2[:, 0] = BIG
    Wu2[~act] = 0.0
    Wi2[~act] = 0.0
    Wc2[~act] = BIG

    coef = np.zeros((B, 3 * GROUPS, 512), np.float32)
    for j in range(GROUPS):
        c1 = slice(64 * j, 64 * j + 64)
        c2 = slice(256 + 64 * j, 256 + 64 * j + 64)
        coef[:, 3 * j + 0, c1] = Wu1
        coef[:, 3 * j + 1, c1] = Wi1
        coef[:, 3 * j + 2, c1] = Wc1
        coef[:, 3 * j + 0, c2] = Wu2
        coef[:, 3 * j + 1, c2] = Wi2
        coef[:, 3 * j + 2, c2] = Wc2
    return coef


def _host_qdata(t_coords, x_coords):
    """(B, 12, ST/2 * 1024) stationary operands: row 3j+(0,1,2) = (u, it, 1),
    column sp*1024 + 512*h + 128*g + m.

    Query assignment q(sp, h, g, j, m) = sp*4096 + h*2048 + m*16 + g*4 + j, so
    each supertile-pair's partition-major store walk (m, h, (g j), k) writes
    one monotonically contiguous 1MB HBM range. The whole batch's qd is one
    flat 384KB contiguous load."""
    it = np.float32(1.0) / (t_coords.reshape(B, NQ) + EPS)
    u = x_coords.reshape(B, NQ) * it

    def lay(v):
        # (b, sp, h, m, g, j) -> [b, sp, j, (h, g, m)]
        v = v.reshape(B, ST_PER_BATCH // 2, 2, CHUNK, SUPER, GROUPS)
        return np.transpose(v, (0, 1, 5, 2, 4, 3)).reshape(
            B, ST_PER_BATCH // 2, GROUPS, 2 * SUPER * CHUNK)

    qd = np.empty((B, ST_PER_BATCH // 2, 3 * GROUPS, 2 * SUPER * CHUNK), np.float32)
    qd[:, :, 0::3, :] = lay(u)
    qd[:, :, 1::3, :] = lay(it)
    qd[:, :, 2::3, :] = 1.0
    # (b, sp, row, col) -> (b, row, sp*1024 + col)
    return np.ascontiguousarray(qd.transpose(0, 2, 1, 3).reshape(
        B, 3 * GROUPS, (ST_PER_BATCH // 2) * 2 * SUPER * CHUNK))


def _build_nc(repeat=1, sp_per_store=SP_PER_STORE):
    """repeat>1 compiles a timing variant whose body re-runs the full
    load->compute->store pipeline `repeat` times (same I/O tensors), so the
    slope over `repeat` isolates steady-state per-execution device time
    from the axon per-dispatch RPC overhead. sp_per_store: supertile-pairs
    (1MB of output) per store DMA."""
    import concourse.bacc as bacc
    import concourse.mybir as mybir
    import concourse.tile as tile

    nc = bacc.Bacc(None, target_bir_lowering=False, debug=False)
    # float32r: same f32 bytes, but the PE streams it at 1 cycle/row for
    # moving dims >= 256 (vs 4 cycles/row for plain fp32's two half-speed
    # passes) — 4x matmul throughput at (reduced, tf32-like) multiply
    # precision, well inside the 2e-2 gate. (bf16 operands were measured
    # identical in speed — loads hide in DMA gaps — so f32r keeps the 78x
    # precision margin for free.)
    f32r = mybir.dt.float32r
    qd_d = nc.declare_dram_parameter(
        "qd", [B_PER_CORE, 3 * GROUPS,
               (ST_PER_BATCH // 2) * 2 * SUPER * CHUNK],
        f32r, isOutput=False)
    cf_d = nc.declare_dram_parameter(
        "cf", [B_PER_CORE, 3 * GROUPS, 512], f32r, isOutput=False)
    out_d = nc.declare_dram_parameter(
        "out", [B_PER_CORE, NQ, K], mybir.dt.float32, isOutput=True)

    f32 = mybir.dt.float32
    with tile.TileContext(nc) as tc:
        with (
            tc.tile_pool(name="cf", bufs=1) as cfp,
            tc.tile_pool(name="qd", bufs=6) as qdp,
            tc.tile_pool(name="ps", bufs=4, space="PSUM") as psp,
            tc.tile_pool(name="p2", bufs=6) as p2p,
            tc.tile_pool(name="ot",
                         bufs=(OT_BUFS or
                               {1: 8, 2: 6, 4: 3, 8: 2}[sp_per_store])) as otp,
        ):
            cft = []
            for b in range(B_PER_CORE):
                c = cfp.tile([3 * GROUPS, 512], f32r, tag=f"cf{b}")
                (nc.sync if LOADS_ON_SP else nc.gpsimd).dma_start(
                    c[:], cf_d[b])
                cft.append(c)
            for rep in range(repeat):
              for b in range(B_PER_CORE):
                for spg in range(ST_PER_BATCH // 2 // sp_per_store):
                    ot = otp.tile(
                        [128, sp_per_store * 2 * SUPER, 256], f32)
                    for si in range(sp_per_store):
                        sp = spg * sp_per_store + si
                        # per-sp 48KB loads (4KB descriptors) ride SWDGE on
                        # the otherwise-idle Pool engine so a store's
                        # semaphore wait on SP can't head-of-line block them
                        # (transfers still share the one SDMA array)
                        o0 = sp * 2 * SUPER * CHUNK
                        qdt = qdp.tile([3 * GROUPS, 2 * SUPER * CHUNK], f32r)
                        (nc.sync if LOADS_ON_SP else nc.gpsimd).dma_start(
                            qdt[:], qd_d[b][:, o0:o0 + 2 * SUPER * CHUNK])
                        for h in range(2):
                            # two 2-bank PSUM tiles per half: deeper PE
                            # pipelining and finer ACT/STT quanta that
                            # overlap the stores.
                            for half in range(2):
                                ps = psp.tile([128, 2, 512], f32)
                                for g2 in range(2):
                                    g = half * 2 + g2
                                    nc.tensor.matmul(
                                        ps[:, g2, :],
                                        qdt[:, 512 * h + 128 * g:
                                            512 * h + 128 * (g + 1)],
                                        cft[b][:],
                                        start=True, stop=True,
                                    )
                                p2 = p2p.tile([128, 2, 256], f32)
                                nc.scalar.activation(
                                    p2[:], ps[:, :, 256:512],
                                    mybir.ActivationFunctionType.Relu,
                                    scale=-1.0)
                                c0 = 8 * si + 4 * h + 2 * half
                                nc.vector.scalar_tensor_tensor(
                                    out=ot[:, c0:c0 + 2],
                                    in0=ps[:, :, 0:256], scalar=0.0,
                                    in1=p2[:],
                                    op0=mybir.AluOpType.min,
                                    op1=mybir.AluOpType.subtract)
                    # one contiguous sp_per_store-MB store per group
                    q0 = spg * sp_per_store * 2 * Q_SUPER
                    sh = sp_per_store * 2
                    dst = out_d[b, q0:q0 + sh * Q_SUPER, :].rearrange(
                        "(s m c) k -> m s c k", s=sh, c=SUPER * GROUPS)
                    src = ot[:].rearrange(
                        "m (s g) (j k) -> m s (g j) k", s=sh, k=K)
                    eng = (nc.gpsimd if (DUAL_RING and spg % 2) else nc.sync)
                    eng.dma_start(dst, src)
    nc.compile()
    return nc


def _get_compiled(repeat=1):
    key = (repeat, SP_PER_STORE, DUAL_RING, OT_BUFS, LOADS_ON_SP)
    if key not in _COMPILED:
        _COMPILED[key] = _build_nc(repeat, SP_PER_STORE)
    return _COMPILED[key]


def device_in_maps(inputs):
    xs = _f32(inputs["xs"])
    ks = _f32(inputs["ks"])
    ks_v = _f32(inputs["ks_v"])
    ks_p = _f32(inputs["ks_p"])
    mask = _f32(inputs["pieces_mask"])
    t_coords = _f32(inputs["t_coords"])
    x_coords = _f32(inputs["x_coords"])

    sl, sr = _host_wave_speeds(xs, ks, ks_v, ks_p)
    coef = _host_coef(xs, mask, sl, sr)
    qd = _host_qdata(t_coords, x_coords)
    return [
        {
            "qd": np.ascontiguousarray(qd[c * B_PER_CORE:(c + 1) * B_PER_CORE]),
            "cf": np.ascontiguousarray(coef[c * B_PER_CORE:(c + 1) * B_PER_CORE]),
        }
        for c in range(N_CORES)
    ]


def run(inputs, trace=False):
    from concourse.bass_utils import run_bass_kernel_spmd

    nc = _get_compiled()
    in_maps = device_in_maps(inputs)
    res = None
    for attempt in range(3):
        try:
            res = run_bass_kernel_spmd(
                nc, in_maps, core_ids=list(range(N_CORES)), trace=trace)
            break
        except Exception:
            if attempt == 2:
                raise
            import time as _time
            _time.sleep(2.0)
    out = np.empty((B, NT, NX, K), np.float32)
    for c in range(N_CORES):
        out[c * B_PER_CORE:(c + 1) * B_PER_CORE] = (
            res.results[c]["out"].reshape(B_PER_CORE, NT, NX, K))
    return out, res


def kernel(**inputs):
    out, _ = run(inputs, trace=False)
    return out

